# revision 12
# baseline (speedup 1.0000x reference)
"""Circulant 1x1 conv (nn_Circulant1x1Conv) as a Trainium2 Bass kernel.

Math: per spatial position r (N = batch*h*w rows):
    y[r, s*C + n] = irfft(rfft(x[r, :]) * cf[s])[n]   (circular convolution)
i.e. Y(N, 2048) = X(N, 512) @ W(512, 2048) with block-circulant W.

CRT factorization (this kernel): t^512 - 1 = (t^256 - 1)(t^256 + 1), so each
512-point circular conv splits into a cyclic-256 and a negacyclic-256 conv on
the half-sums a = x_lo + x_hi, b = x_lo - x_hi:
    u_s = a @ U_s   (U_s cyclic from ca_s = c_lo + c_hi)
    v_s = b @ V_s   (V_s negacyclic from cb_s = c_lo - c_hi)
    y_lo = (u_s + v_s)/2,  y_hi = (u_s - v_s)/2
This HALVES the tensor-engine MACs (2 x 256^2 vs 512^2 per stack). The /2 is
folded into the weights on host; the reconstruction add/sub replaces the
PSUM->SBUF copies (same element count) on the DVE + Pool engines.

I/O is fp16 (tolerance is 2e-2; fp16 end-to-end lands ~1e-3), which also
halves HBM traffic: in 4+1 MB, out 16 MB per core vs 44 MB for fp32.

Sharding: data-parallel over batch, 4 batches per core x 8 cores.

Device output layout: row = mu*256 + hb*128 + p with mu = s*2 + h, hb = lo/hi
(channel = s*512 + hb*256 + h*128 + p); the host permutes back.
"""

import numpy as np

SIZE = 512          # channels C (circulant size)
HALF = SIZE // 2    # CRT half size = 256
NSTACK = 4
BATCH = 32
HW = 32 * 32
N_CORES = 8
BPC = BATCH // N_CORES          # batches per core = 4
COLS = BPC * HW                 # moving free dim per core = 4096
M_OUT = NSTACK * SIZE           # output channels = 2048
P = 128
KC = HALF // P                  # contraction chunks = 2
MU = NSTACK * HALF // P         # u (and v) output row tiles = 8
NF = 512                        # matmul moving free dim (1 PSUM bank fp32)
JW = 2 * NF                     # columns per group = 1024 (one 2-bank psum)
JJ = COLS // JW                 # column groups = 4

_CACHE = {}


def _build_nc():
    import concourse.bacc as bacc
    import concourse.tile as tile
    from concourse import mybir

    io_dt = mybir.dt.float16
    f32 = mybir.dt.float32

    nc = bacc.Bacc("TRN2", name="circulant_crt")
    a = nc.dram_tensor("a", [HALF, COLS], io_dt, kind="ExternalInput")
    b = nc.dram_tensor("b", [HALF, COLS], io_dt, kind="ExternalInput")
    wu = nc.dram_tensor("wu", [HALF, MU * P], io_dt, kind="ExternalInput")
    wv = nc.dram_tensor("wv", [HALF, MU * P], io_dt, kind="ExternalInput")
    out = nc.dram_tensor("out", [M_OUT, COLS], io_dt, kind="ExternalOutput")

    with tile.TileContext(nc) as tc:
        with (
            tc.tile_pool(name="ain", bufs=1) as ip,
            tc.tile_pool(name="win", bufs=1) as wp,
            tc.tile_pool(name="outp", bufs=6) as op,
            tc.tile_pool(name="ps", bufs=4, space="PSUM") as pp,
        ):
            a_sb = ip.tile([P, KC, COLS], io_dt)
            b_sb = ip.tile([P, KC, COLS], io_dt)
            wu_sb = wp.tile([P, KC, MU * P], io_dt)
            wv_sb = wp.tile([P, KC, MU * P], io_dt)

            def ld(dst, src):
                nc.sync.dma_start(
                    out=dst, in_=src.rearrange("(k p) c -> p k c", p=P))

            # Input order on the sync HWDGE queue, k-split at the head so the
            # first iteration's k0 matmuls (then k1) un-gate as early as
            # possible: weights k0, a/b first-group k0, weights k1, a/b
            # first-group k1, then the remaining column groups.
            nc.sync.dma_start(out=wu_sb[:, 0, :], in_=wu[0:P, :])
            nc.sync.dma_start(out=wv_sb[:, 0, :], in_=wv[0:P, :])
            nc.sync.dma_start(out=a_sb[:, 0, 0:JW], in_=a[0:P, 0:JW])
            nc.sync.dma_start(out=b_sb[:, 0, 0:JW], in_=b[0:P, 0:JW])
            nc.sync.dma_start(out=wu_sb[:, 1, :], in_=wu[P:, :])
            nc.sync.dma_start(out=wv_sb[:, 1, :], in_=wv[P:, :])
            nc.sync.dma_start(out=a_sb[:, 1, 0:JW], in_=a[P:, 0:JW])
            nc.sync.dma_start(out=b_sb[:, 1, 0:JW], in_=b[P:, 0:JW])
            for jj in range(1, JJ):
                ld(a_sb[:, :, jj * JW:(jj + 1) * JW], a[:, jj * JW:(jj + 1) * JW])
                ld(b_sb[:, :, jj * JW:(jj + 1) * JW], b[:, jj * JW:(jj + 1) * JW])

            # HAM warmup: dummy matmuls on a memset scratch tile. Gating on a
            # memset (instead of the first weight DMA) lets the warmup start
            # during the framework preamble, fully overlapped with the input
            # stream, so the PE clock is ramped when the real matmuls begin.
            scratch = wp.tile([P, NF], io_dt)
            nc.gpsimd.memset(scratch[:, :], 0.0)
            for i in range(10):
                wps = pp.tile([P, JW], f32, tag="ps", name=f"warm_{i}")
                nc.tensor.matmul(wps[:, 0:NF], scratch[:, 0:P],
                                 scratch[:, 0:NF], start=True, stop=True)

            # Main sweep: column groups outer (so compute tracks the a/b
            # input stream), u/v row tiles inner. Each iteration fills one
            # (ps_u, ps_v) 2-bank pair, casts both to fp16 staging (PSUM has
            # one read port per engine and GPSIMD can't touch it, so Act and
            # DVE split the 1-input evacuation casts), and DMAs u,v out on
            # the sync queue. The y_lo/y_hi = (u +- v) reconstruction happens
            # on host during unshard — same output bytes either way.
            #
            # Matmuls are k-outer so back-to-back matmuls share a stationary
            # (halves LD_WEIGHTS traffic).
            for jj in range(JJ):
                for mu in range(MU):
                    ps_u = pp.tile([P, JW], f32, tag="ps", name=f"psu_{jj}_{mu}")
                    ps_v = pp.tile([P, JW], f32, tag="ps", name=f"psv_{jj}_{mu}")
                    for ps, w_sb, x_sb in ((ps_u, wu_sb, a_sb),
                                           (ps_v, wv_sb, b_sb)):
                        for k in range(KC):
                            for cc in range(2):
                                col = jj * JW + cc * NF
                                ps_slice = ps[:, cc * NF:(cc + 1) * NF]
                                nc.tensor.matmul(
                                    ps_slice,
                                    w_sb[:, k, mu * P:(mu + 1) * P],
                                    x_sb[:, k, col:col + NF],
                                    start=(k == 0), stop=(k == KC - 1))

                    st = op.tile([P, 2, JW], io_dt, tag="osb",
                                 name=f"st_{jj}_{mu}")
                    it = jj * MU + mu
                    if it == JJ * MU - 1:
                        # Last iteration: drain each of u/v with split copies
                        # (Act + DVE in parallel) and its own DMA so the
                        # kernel tail after the final matmul is minimal.
                        for hb, ps in ((0, ps_u), (1, ps_v)):
                            nc.scalar.copy(out=st[:, hb, 0:NF],
                                           in_=ps[:, 0:NF])
                            nc.vector.tensor_copy(out=st[:, hb, NF:JW],
                                                  in_=ps[:, NF:JW])
                            row0 = mu * 2 * P + hb * P
                            nc.sync.dma_start(
                                out=out[row0:row0 + P,
                                        jj * JW:(jj + 1) * JW],
                                in_=st[:, hb, :])
                        continue
                    # Alternate which engine takes u vs v for balance
                    # (Act ~1.04us, DVE ~1.17us per [128,1024] cast).
                    if it % 2 == 0:
                        nc.scalar.copy(out=st[:, 0, :], in_=ps_u[:, :])
                        nc.vector.tensor_copy(out=st[:, 1, :], in_=ps_v[:, :])
                    else:
                        nc.vector.tensor_copy(out=st[:, 0, :], in_=ps_u[:, :])
                        nc.scalar.copy(out=st[:, 1, :], in_=ps_v[:, :])
                    nc.sync.dma_start(
                        out=out[mu * 2 * P:(mu + 1) * 2 * P,
                                jj * JW:(jj + 1) * JW]
                        .rearrange("(hb p) c -> p hb c", hb=2),
                        in_=st[:])
    nc.compile()
    return nc


def get_nc():
    if "nc" not in _CACHE:
        _CACHE["nc"] = _build_nc()
    return _CACHE["nc"]


def build_weights(c_f):
    """(NSTACK, SIZE//2+1, 2) rfft coeffs -> (wu, wv) each (HALF, MU*P) fp32.

    wu[:, (s*2+h)*128 + p] = 0.5 * U_s[:, h*128 + p] with U_s the cyclic-256
    matrix of ca_s; wv likewise with the negacyclic V_s of cb_s.
    """
    c_f = np.asarray(c_f, np.float32)
    cf = c_f[..., 0].astype(np.float64) + 1j * c_f[..., 1].astype(np.float64)
    c = np.fft.irfft(cf, n=SIZE, axis=-1)            # (NSTACK, SIZE) float64
    ca = c[:, :HALF] + c[:, HALF:]
    cb = c[:, :HALF] - c[:, HALF:]
    d = np.arange(HALF)[None, :] - np.arange(HALF)[:, None]   # n - k
    idx = d % HALF
    sign = np.where(d >= 0, 1.0, -1.0)
    wu = np.empty((HALF, MU * P), np.float32)
    wv = np.empty((HALF, MU * P), np.float32)
    for s in range(NSTACK):
        wu[:, s * HALF:(s + 1) * HALF] = 0.5 * ca[s][idx]
        wv[:, s * HALF:(s + 1) * HALF] = 0.5 * cb[s][idx] * sign
    return wu, wv


def make_in_maps(x, c_f):
    x = np.asarray(x, np.float32)
    wu, wv = build_weights(c_f)
    wu16 = wu.astype(np.float16)
    wv16 = wv.astype(np.float16)
    in_maps = []
    for i in range(N_CORES):
        xs = (x[i * BPC:(i + 1) * BPC]
              .reshape(BPC, SIZE, HW)
              .transpose(1, 0, 2)
              .reshape(SIZE, COLS))
        a = (xs[:HALF] + xs[HALF:]).astype(np.float16)
        b = (xs[:HALF] - xs[HALF:]).astype(np.float16)
        in_maps.append({"a": np.ascontiguousarray(a),
                        "b": np.ascontiguousarray(b),
                        "wu": wu16, "wv": wv16})
    return in_maps


def dev_to_chan(dev_out):
    """Device-order u/v (M_OUT, COLS) -> channel-order y (M_OUT, COLS).

    Device row = s*512 + h*256 + hb*128 + p with hb in {u, v}; the CRT
    reconstruction y_lo = u + v, y_hi = u - v (the /2 is folded into the
    weights) happens here, and channel = s*512 + lohi*256 + h*128 + p.
    """
    o = dev_out.reshape(NSTACK, 2, 2, P, COLS)       # (s, h, uv, p, c)
    u = o[:, :, 0]
    v = o[:, :, 1]
    y = np.stack([u + v, u - v], axis=1)             # (s, lohi, h, p, c)
    return y.reshape(M_OUT, COLS)


def assemble_output(per_core_outs):
    """list of 8 (M_OUT, COLS) fp16 device-order -> (BATCH, M_OUT, 32, 32) f32"""
    parts = []
    for o in per_core_outs:
        oc = dev_to_chan(np.asarray(o).astype(np.float32))
        parts.append(oc.reshape(M_OUT, BPC, HW).transpose(1, 0, 2))
    out = np.concatenate(parts, axis=0)               # (BATCH, M_OUT, HW)
    return np.ascontiguousarray(out.reshape(BATCH, M_OUT, 32, 32), np.float32)


def run(x, c_f, **run_kwargs):
    """Returns (full_output, BassKernelResults)."""
    from concourse.bass_utils import run_bass_kernel_spmd
    nc = get_nc()
    in_maps = make_in_maps(x, c_f)
    res = run_bass_kernel_spmd(nc, in_maps, core_ids=list(range(N_CORES)),
                               **run_kwargs)
    out = assemble_output([r["out"] for r in res.results])
    return out, res


def kernel(input, c_f):
    out, _ = run(input, c_f)
    return out


# revision 13
# speedup vs baseline: 1.0257x; 1.0257x over previous
"""Circulant 1x1 conv (nn_Circulant1x1Conv) as a Trainium2 Bass kernel.

Math: per spatial position r (N = batch*h*w rows):
    y[r, s*C + n] = irfft(rfft(x[r, :]) * cf[s])[n]   (circular convolution)
i.e. Y(N, 2048) = X(N, 512) @ W(512, 2048) with block-circulant W.

CRT factorization (this kernel): t^512 - 1 = (t^256 - 1)(t^256 + 1), so each
512-point circular conv splits into a cyclic-256 and a negacyclic-256 conv on
the half-sums a = x_lo + x_hi, b = x_lo - x_hi:
    u_s = a @ U_s   (U_s cyclic from ca_s = c_lo + c_hi)
    v_s = b @ V_s   (V_s negacyclic from cb_s = c_lo - c_hi)
    y_lo = (u_s + v_s)/2,  y_hi = (u_s - v_s)/2
This HALVES the tensor-engine MACs (2 x 256^2 vs 512^2 per stack). The /2 is
folded into the weights on host; the reconstruction add/sub replaces the
PSUM->SBUF copies (same element count) on the DVE + Pool engines.

I/O is fp16 (tolerance is 2e-2; fp16 end-to-end lands ~1e-3), which also
halves HBM traffic: in 4+1 MB, out 16 MB per core vs 44 MB for fp32.

Sharding: data-parallel over batch, 4 batches per core x 8 cores.

Device output layout: row = mu*256 + hb*128 + p with mu = s*2 + h, hb = lo/hi
(channel = s*512 + hb*256 + h*128 + p); the host permutes back.
"""

import numpy as np

SIZE = 512          # channels C (circulant size)
HALF = SIZE // 2    # CRT half size = 256
NSTACK = 4
BATCH = 32
HW = 32 * 32
N_CORES = 8
BPC = BATCH // N_CORES          # batches per core = 4
COLS = BPC * HW                 # moving free dim per core = 4096
M_OUT = NSTACK * SIZE           # output channels = 2048
P = 128
KC = HALF // P                  # contraction chunks = 2
MU = NSTACK * HALF // P         # u (and v) output row tiles = 8
NF = 512                        # matmul moving free dim (1 PSUM bank fp32)
JW = 2 * NF                     # columns per group = 1024 (one 2-bank psum)
JJ = COLS // JW                 # column groups = 4

_CACHE = {}


def _build_nc():
    import concourse.bacc as bacc
    import concourse.tile as tile
    from concourse import mybir

    io_dt = mybir.dt.float16
    f32 = mybir.dt.float32

    nc = bacc.Bacc("TRN2", name="circulant_crt")
    a = nc.dram_tensor("a", [HALF, COLS], io_dt, kind="ExternalInput")
    b = nc.dram_tensor("b", [HALF, COLS], io_dt, kind="ExternalInput")
    wu = nc.dram_tensor("wu", [HALF, MU * P], io_dt, kind="ExternalInput")
    wv = nc.dram_tensor("wv", [HALF, MU * P], io_dt, kind="ExternalInput")
    out = nc.dram_tensor("out", [M_OUT, COLS], io_dt, kind="ExternalOutput")

    with tile.TileContext(nc) as tc:
        with (
            tc.tile_pool(name="ain", bufs=1) as ip,
            tc.tile_pool(name="win", bufs=1) as wp,
            tc.tile_pool(name="outp", bufs=6) as op,
            tc.tile_pool(name="ps", bufs=4, space="PSUM") as pp,
        ):
            a_sb = ip.tile([P, KC, COLS], io_dt)
            b_sb = ip.tile([P, KC, COLS], io_dt)
            wu_sb = wp.tile([P, KC, MU * P], io_dt)
            wv_sb = wp.tile([P, KC, MU * P], io_dt)

            def ld(dst, src):
                nc.sync.dma_start(
                    out=dst, in_=src.rearrange("(k p) c -> p k c", p=P))

            # Input order on the sync HWDGE queue: weights, then the first
            # column group of a/b (so the first real iteration has all its
            # operands at once — a partially-landed group stalls the PE
            # mid-ramp and the HAM re-throttles), then the rest by group.
            ld(wu_sb[:, :, :], wu[:, :])
            ld(wv_sb[:, :, :], wv[:, :])
            for jj in range(JJ):
                ld(a_sb[:, :, jj * JW:(jj + 1) * JW], a[:, jj * JW:(jj + 1) * JW])
                ld(b_sb[:, :, jj * JW:(jj + 1) * JW], b[:, jj * JW:(jj + 1) * JW])

            # HAM warmup: dummy matmuls on a memset scratch tile. Gating on a
            # memset (instead of the first weight DMA) lets the warmup start
            # during the framework preamble (~7.4us), fully overlapped with
            # the input stream; 13 matmuls at ~430ns bridge until the first
            # a/b group lands (~13us) so the PE never idles once ramped.
            scratch = wp.tile([P, NF], io_dt)
            nc.gpsimd.memset(scratch[:, :], 0.0)
            for i in range(13):
                wps = pp.tile([P, JW], f32, tag="ps", name=f"warm_{i}")
                nc.tensor.matmul(wps[:, 0:NF], scratch[:, 0:P],
                                 scratch[:, 0:NF], start=True, stop=True)

            # Main sweep: column groups outer (so compute tracks the a/b
            # input stream), u/v row tiles inner. Each iteration fills one
            # (ps_u, ps_v) 2-bank pair, casts both to fp16 staging (PSUM has
            # one read port per engine and GPSIMD can't touch it, so Act and
            # DVE split the 1-input evacuation casts), and DMAs u,v out on
            # the sync queue. The y_lo/y_hi = (u +- v) reconstruction happens
            # on host during unshard — same output bytes either way.
            #
            # Matmuls are k-outer so back-to-back matmuls share a stationary
            # (halves LD_WEIGHTS traffic).
            for jj in range(JJ):
                for mu in range(MU):
                    ps_u = pp.tile([P, JW], f32, tag="ps", name=f"psu_{jj}_{mu}")
                    ps_v = pp.tile([P, JW], f32, tag="ps", name=f"psv_{jj}_{mu}")
                    for ps, w_sb, x_sb in ((ps_u, wu_sb, a_sb),
                                           (ps_v, wv_sb, b_sb)):
                        for k in range(KC):
                            for cc in range(2):
                                col = jj * JW + cc * NF
                                ps_slice = ps[:, cc * NF:(cc + 1) * NF]
                                nc.tensor.matmul(
                                    ps_slice,
                                    w_sb[:, k, mu * P:(mu + 1) * P],
                                    x_sb[:, k, col:col + NF],
                                    start=(k == 0), stop=(k == KC - 1))

                    st = op.tile([P, 2, JW], io_dt, tag="osb",
                                 name=f"st_{jj}_{mu}")
                    it = jj * MU + mu
                    if it == JJ * MU - 1:
                        # Last iteration: drain each of u/v with split copies
                        # (Act + DVE in parallel) and its own DMA so the
                        # kernel tail after the final matmul is minimal.
                        for hb, ps in ((0, ps_u), (1, ps_v)):
                            nc.scalar.copy(out=st[:, hb, 0:NF],
                                           in_=ps[:, 0:NF])
                            nc.vector.tensor_copy(out=st[:, hb, NF:JW],
                                                  in_=ps[:, NF:JW])
                            row0 = mu * 2 * P + hb * P
                            nc.sync.dma_start(
                                out=out[row0:row0 + P,
                                        jj * JW:(jj + 1) * JW],
                                in_=st[:, hb, :])
                        continue
                    # Alternate which engine takes u vs v for balance
                    # (Act ~1.04us, DVE ~1.17us per [128,1024] cast).
                    if it % 2 == 0:
                        nc.scalar.copy(out=st[:, 0, :], in_=ps_u[:, :])
                        nc.vector.tensor_copy(out=st[:, 1, :], in_=ps_v[:, :])
                    else:
                        nc.vector.tensor_copy(out=st[:, 0, :], in_=ps_u[:, :])
                        nc.scalar.copy(out=st[:, 1, :], in_=ps_v[:, :])
                    nc.sync.dma_start(
                        out=out[mu * 2 * P:(mu + 1) * 2 * P,
                                jj * JW:(jj + 1) * JW]
                        .rearrange("(hb p) c -> p hb c", hb=2),
                        in_=st[:])
    nc.compile()
    return nc


def get_nc():
    if "nc" not in _CACHE:
        _CACHE["nc"] = _build_nc()
    return _CACHE["nc"]


def build_weights(c_f):
    """(NSTACK, SIZE//2+1, 2) rfft coeffs -> (wu, wv) each (HALF, MU*P) fp32.

    wu[:, (s*2+h)*128 + p] = 0.5 * U_s[:, h*128 + p] with U_s the cyclic-256
    matrix of ca_s; wv likewise with the negacyclic V_s of cb_s.
    """
    c_f = np.asarray(c_f, np.float32)
    cf = c_f[..., 0].astype(np.float64) + 1j * c_f[..., 1].astype(np.float64)
    c = np.fft.irfft(cf, n=SIZE, axis=-1)            # (NSTACK, SIZE) float64
    ca = c[:, :HALF] + c[:, HALF:]
    cb = c[:, :HALF] - c[:, HALF:]
    d = np.arange(HALF)[None, :] - np.arange(HALF)[:, None]   # n - k
    idx = d % HALF
    sign = np.where(d >= 0, 1.0, -1.0)
    wu = np.empty((HALF, MU * P), np.float32)
    wv = np.empty((HALF, MU * P), np.float32)
    for s in range(NSTACK):
        wu[:, s * HALF:(s + 1) * HALF] = 0.5 * ca[s][idx]
        wv[:, s * HALF:(s + 1) * HALF] = 0.5 * cb[s][idx] * sign
    return wu, wv


def make_in_maps(x, c_f):
    x = np.asarray(x, np.float32)
    wu, wv = build_weights(c_f)
    wu16 = wu.astype(np.float16)
    wv16 = wv.astype(np.float16)
    in_maps = []
    for i in range(N_CORES):
        xs = (x[i * BPC:(i + 1) * BPC]
              .reshape(BPC, SIZE, HW)
              .transpose(1, 0, 2)
              .reshape(SIZE, COLS))
        a = (xs[:HALF] + xs[HALF:]).astype(np.float16)
        b = (xs[:HALF] - xs[HALF:]).astype(np.float16)
        in_maps.append({"a": np.ascontiguousarray(a),
                        "b": np.ascontiguousarray(b),
                        "wu": wu16, "wv": wv16})
    return in_maps


def dev_to_chan(dev_out):
    """Device-order u/v (M_OUT, COLS) -> channel-order y (M_OUT, COLS).

    Device row = s*512 + h*256 + hb*128 + p with hb in {u, v}; the CRT
    reconstruction y_lo = u + v, y_hi = u - v (the /2 is folded into the
    weights) happens here, and channel = s*512 + lohi*256 + h*128 + p.
    """
    o = dev_out.reshape(NSTACK, 2, 2, P, COLS)       # (s, h, uv, p, c)
    u = o[:, :, 0]
    v = o[:, :, 1]
    y = np.stack([u + v, u - v], axis=1)             # (s, lohi, h, p, c)
    return y.reshape(M_OUT, COLS)


def assemble_output(per_core_outs):
    """list of 8 (M_OUT, COLS) fp16 device-order -> (BATCH, M_OUT, 32, 32) f32"""
    parts = []
    for o in per_core_outs:
        oc = dev_to_chan(np.asarray(o).astype(np.float32))
        parts.append(oc.reshape(M_OUT, BPC, HW).transpose(1, 0, 2))
    out = np.concatenate(parts, axis=0)               # (BATCH, M_OUT, HW)
    return np.ascontiguousarray(out.reshape(BATCH, M_OUT, 32, 32), np.float32)


def run(x, c_f, **run_kwargs):
    """Returns (full_output, BassKernelResults)."""
    from concourse.bass_utils import run_bass_kernel_spmd
    nc = get_nc()
    in_maps = make_in_maps(x, c_f)
    res = run_bass_kernel_spmd(nc, in_maps, core_ids=list(range(N_CORES)),
                               **run_kwargs)
    out = assemble_output([r["out"] for r in res.results])
    return out, res


def kernel(input, c_f):
    out, _ = run(input, c_f)
    return out
